# revision 1
# baseline (speedup 1.0000x reference)
"""Trainium2 Bass kernel for nn_LinearRNN (B=16, T=4096, D_in=256, H=512, D_out=256).

  xp = x @ W_in.T                       [B, T, H]
  h_t = xp_t + h_{t-1} @ W_h.T          (W_h is diagonal -> elementwise scan)
  out = hs @ W_out.T                    [B, T, D_out]

Batch data-parallel over 8 cores (2 batch rows per core). Default mode 'hl8':

  - matmul1 runs as fp8e4m3 hi/lo DoubleRow: x and W_in each split into
    (hi, lo*16) fp8 planes; 3 DoubleRow instrs per 256-contraction at
    0.5 cyc/row = 1.5 cyc vs bf16's 2.0, at ~bf16 accuracy (7-bit mantissa).
  - the recurrence runs on VectorE tensor_tensor_scan over super-chunks of
    1024 columns (xp in 2-bank PSUM tiles) with the per-h decay broadcast
    from a [128,1] column; carry chains via the previous tile's last column.
  - channels are host-permuted by decay: the 256 largest-u ("hot") channels
    scan to bf16, the 256 small-u ("cold") channels scan to fp8 (their hs
    variance is small and the fp8 carry error decays immediately). matmul2
    contracts hot via two bf16 ktiles + cold via one fp8 DoubleRow pair ->
    2.5 cyc/col instead of 4.0.
  - ScalarE copies PSUM->SBUF bf16, output DMAs back as [b, o, t] bf16 and
    the host transposes/rescales to fp32 [b, t, o].
  - PE warmup matmuls ramp the tensor-engine p-state during the DMA fill;
    scales SWI/WOS are powers of two (exact), divided out on the host.

Legacy modes 'f32r'/'bf16'/'f32' keep the original single-precision path.
"""
from contextlib import ExitStack

import numpy as np

import concourse.bass as bass
import concourse.mybir as mybir
import concourse.tile as tile
from concourse import bacc
from concourse.bass_utils import run_bass_kernel_spmd

B, T, D_IN, HID, D_OUT = 16, 4096, 256, 512, 256
NCORES = 8
BPC = B // NCORES          # batch rows per core
TC = 512                   # t-chunk (PSUM bank = 512 fp32)
NCH = T // TC
ND = D_IN // 128           # 2  d-blocks
NH = HID // 128            # 4  h-blocks
NO = D_OUT // 128          # 2  o-blocks

SWI = 16.0                 # hl8: scale on W_in (cold hs must stay under the
                           # device fp8e4 max of 240: |hs_cold| < ~135 at x16)
WOS = 16.0                 # hl8: scale on W_out (out scaled by SWI*WOS, host divides)
N_HOT = 256                # channels (by largest decay u) kept bf16 in matmul2;
                           # the cold half rides fp8 DoubleRow (u<~0.5 ->
                           # small hs variance and negligible carry error)

MODE_DEFAULT = "hl8"

# schedule/tuning knobs (read by _build; cache key includes them)
CFG = dict(sched="pipe1x", xp_bufs=3, op_bufs=2, hs_bufs=4, hc_bufs=4,
           x_piece=1024, out_piece=512, wu_n=14,
           mm2_order="o", out_q="sp", split_last=False,
           head_small=0, tail_small=0, last_h=False, par_tail=False)

_cache: dict = {}


# ---------------------------------------------------------------- hl8 build --

def _build_hl8() -> bass.Bass:
    f32 = mybir.dt.float32
    bf = mybir.dt.bfloat16
    fp8 = mybir.dt.float8e4
    DR = mybir.MatmulPerfMode.DoubleRow

    nc = bacc.Bacc(None, target_bir_lowering=False)

    # x planes per batch row: [128, 4, T]: 0=hi(d=p) 1=lo(d=p) 2=hi(d=128+p) 3=lo(d=128+p)
    xq = nc.declare_dram_parameter("xq", [BPC, 128, 4, T], fp8, isOutput=False)
    # mm1 stationaries packed per h-block: planes j=0:(Whi-d0, Whi-d0/16),
    # j=1:(Whi-d1, Whi-d1/16), j=2:(Wlo-d0/16, Wlo-d1/16)
    wabc = nc.declare_dram_parameter("wabc", [128, NH, 3, 2, 128], fp8,
                                     isOutput=False)
    w_outT = nc.declare_dram_parameter("w_outT", [N_HOT, D_OUT], bf, isOutput=False)
    wo_cold = nc.declare_dram_parameter("wo_cold", [128, 2, D_OUT], fp8,
                                        isOutput=False)
    dcols = nc.declare_dram_parameter("dcols", [128, NH], f32, isOutput=False)
    out = nc.declare_dram_parameter("out", [BPC, D_OUT, T], bf, isOutput=True)

    with tile.TileContext(nc) as tc, ExitStack() as ctx:
        const_pool = ctx.enter_context(tc.tile_pool(name="const", bufs=1))
        x_pool = ctx.enter_context(tc.tile_pool(name="xt", bufs=BPC))
        o_pool = ctx.enter_context(tc.tile_pool(name="ot", bufs=CFG.get("ot_bufs", 6)))
        hs_pool = ctx.enter_context(tc.tile_pool(name="hs", bufs=CFG["hs_bufs"]))
        hs8_pool = ctx.enter_context(
            tc.tile_pool(name="hc", bufs=CFG.get("hc_bufs", 8)))
        xp_psum = ctx.enter_context(
            tc.tile_pool(name="xp", bufs=CFG["xp_bufs"], space=bass.MemorySpace.PSUM))
        op_psum = ctx.enter_context(
            tc.tile_pool(name="op", bufs=CFG["op_bufs"], space=bass.MemorySpace.PSUM))

        # --- PE warmup: ramp the tensor-engine clock while input DMAs land.
        # Matmuls on a zeroed SBUF tile into a rotating PSUM buf; no DMA deps,
        # so they start at t~0 and keep PE continuously busy into real work.
        if CFG.get("wu_n", 0):
            wu = const_pool.tile([128, 256], bf, tag="wu")
            nc.vector.memset(wu[:], 0.0)
            wu_ps = op_psum.tile([128, 256], f32, name="wu", tag="op")
            for _ in range(CFG["wu_n"]):
                nc.tensor.matmul(wu_ps[:], wu[:, 0:128], wu[:],
                                 start=True, stop=True)

        # --- DMAs: first x piece (small, unblocks mm1 soonest), then the mm1
        # weights + decay, then the rest of x interleaved with wo.
        XP_LEN = CFG["x_piece"]
        xt = {}
        for b in range(BPC):
            xt[b] = x_pool.tile([128, 4, T], fp8, name="xt", tag="xt")

        def load_x(b, lo, hi):
            psl = slice(lo, hi)
            nc.sync.dma_start(xt[b][:, :, psl], xq[b, :, :, psl])

        dc = const_pool.tile([128, NH], f32, tag="dc")
        nc.scalar.dma_start(dc[:], dcols[:])
        first_w = 512 if CFG.get("head512", False) else 1024
        load_x(0, 0, first_w)
        wx0 = const_pool.tile([128, 3, 2, 128], fp8, tag="wx0")
        nc.sync.dma_start(wx0[:], wabc[:, 0])
        wxr = const_pool.tile([128, NH - 1, 3, 2, 128], fp8, tag="wxr")
        nc.sync.dma_start(wxr[:], wabc[:, 1:NH])

        def wx(hblk, j):
            if hblk == 0:
                return wx0[:, j, :, :]
            return wxr[:, hblk - 1, j, :, :]
        lo = first_w
        while lo < T:
            hi = min(lo + XP_LEN, T)
            load_x(0, lo, hi)
            lo = hi
        wo = []
        for hblk in range(N_HOT // 128):
            w = const_pool.tile([128, D_OUT], bf, tag=f"wo{hblk}")
            nc.sync.dma_start(w[:], w_outT[hblk * 128:(hblk + 1) * 128, :])
            wo.append(w)
        woc = const_pool.tile([128, 2, D_OUT], fp8, tag="woc")
        nc.sync.dma_start(woc[:], wo_cold[:])
        for b in range(1, BPC):
            for lo in range(0, T, XP_LEN):
                load_x(b, lo, lo + XP_LEN)

        # super-chunks: one scan instruction spans width/TC chunks of the
        # same h-block (just a longer scan; carry via `initial`). The last
        # batch ends with two 512 supers so the post-scan tail is short.
        SC = CFG.get("sc", 1024)
        sup = []   # (b, lo, width)
        for b in range(BPC):
            if CFG.get("sc1536", False):
                widths = [1536, 1536, 1024]
                if b == BPC - 1 and CFG.get("tail512", True):
                    widths = [1536, 1536, 512, 512]
            else:
                widths = [SC] * (T // SC)
                if b == BPC - 1 and CFG.get("tail256", False):
                    widths = [SC] * (T // SC - 1) + [512, 256, 256]
                elif b == BPC - 1 and CFG.get("tail512", True):
                    widths = [SC] * (T // SC - 1) + [512, 512]
                elif b == 0 and CFG.get("head512", False):
                    widths = [512] + [SC] * (T // SC - 1) + [512]
            lo = 0
            for w in widths:
                sup.append((b, lo, w))
                lo += w
        max_w = max(w for _, _, w in sup)
        prev_hot = {}    # (b, lo) -> (bf16 tile [128, 2, SC] (hb0|hb1), width)
        prev_cold = {}   # (b, lo) -> (fp8 tile [128, 2, SC]  (hb2|hb3), width)
        assert N_HOT == 256

        def stage1(b, lo, w):
            """hi/lo-fp8 DoubleRow matmul1 + one scan per (hb, super-chunk)."""
            hot = hs_pool.tile([128, 2, max_w], bf, name="hh", tag="hh")
            cold = hs8_pool.tile([128, 2, max_w], fp8, name="hc", tag="hc")
            prev_hot[(b, lo)] = (hot, w)
            prev_cold[(b, lo)] = (cold, w)
            hb_order = (2, 3, 0, 1) if CFG.get("cold_first", False) else range(NH)
            for hblk in hb_order:
                xp = xp_psum.tile([128, max_w], f32, name="xp", tag="xp")
                for c in range(w // 256):
                    tsl = slice(lo + c * 256, lo + (c + 1) * 256)
                    osl = slice(c * 256, (c + 1) * 256)
                    nc.tensor.matmul(xp[:, osl], wx(hblk, 0),
                                     xt[b][:, 0:2, tsl],
                                     start=True, stop=False, perf_mode=DR)
                    nc.tensor.matmul(xp[:, osl], wx(hblk, 1),
                                     xt[b][:, 2:4, tsl],
                                     start=False, stop=False, perf_mode=DR)
                    nc.tensor.matmul(xp[:, osl], wx(hblk, 2),
                                     xt[b][:, 0:4:2, tsl],
                                     start=False, stop=True, perf_mode=DR)
                pair, seg = divmod(hblk, 2)
                dst = (hot if pair == 0 else cold)[:, seg, 0:w]
                if lo == 0:
                    init = 0.0
                else:
                    prev = prev_hot if pair == 0 else prev_cold
                    pt, pw = prev[(b, plo[b])]
                    init = pt[:, seg, pw - 1:pw]
                nc.vector.tensor_tensor_scan(
                    dst, dc[:, hblk:hblk + 1].to_broadcast((128, w)),
                    xp[:, 0:w], init,
                    op0=mybir.AluOpType.mult, op1=mybir.AluOpType.add)
            plo[b] = lo

        plo = {}   # b -> lo of the batch's most recent super
        ot2 = {}   # oblk -> staging tile for the final two supers

        def stage2(b, slo, off, cw, oblks=range(NO)):
            """matmul2 on the cw-wide chunk at offset off of super slo: hot
            half bf16, cold half fp8 DoubleRow; then PSUM->SBUF bf16 copy +
            out DMA."""
            lo = slo + off
            dma_q = nc.scalar if CFG.get("out_q", "sp") == "act" else nc.sync
            hot = prev_hot[(b, slo)][0]
            cold = prev_cold[(b, slo)][0]
            for oblk in oblks:
                op = op_psum.tile([128, TC], f32, name="op", tag="op")
                osl_o = slice(oblk * 128, (oblk + 1) * 128)
                for c in range(cw // 256):
                    csl = slice(off + c * 256, off + (c + 1) * 256)
                    osl = slice(c * 256, (c + 1) * 256)
                    if CFG.get("cold_first", False):
                        # cold scans run first in each super, so lead the
                        # group with the DR ktile: after the super's last
                        # (hot) scan only one hot ktile remains on PE
                        nc.tensor.matmul(op[:, osl], woc[:, :, osl_o],
                                         cold[:, :, csl],
                                         start=True, stop=False,
                                         perf_mode=DR)
                        nc.tensor.matmul(op[:, osl], wo[0][:, osl_o],
                                         hot[:, 0, csl],
                                         start=False, stop=False)
                        nc.tensor.matmul(op[:, osl], wo[1][:, osl_o],
                                         hot[:, 1, csl],
                                         start=False, stop=True)
                    else:
                        nc.tensor.matmul(op[:, osl], wo[0][:, osl_o],
                                         hot[:, 0, csl],
                                         start=True, stop=False)
                        nc.tensor.matmul(op[:, osl], wo[1][:, osl_o],
                                         hot[:, 1, csl],
                                         start=False, stop=False)
                        nc.tensor.matmul(op[:, osl], woc[:, :, osl_o],
                                         cold[:, :, csl],
                                         start=False, stop=True,
                                         perf_mode=DR)
                t = o_pool.tile([128, TC], bf, name="ot", tag="ot")
                nc.scalar.copy(t[:, 0:cw], op[:, 0:cw])
                dma_q.dma_start(out[b, osl_o, lo:lo + cw], t[:, 0:cw])

        def chunks_of(w):
            offs = []
            o = 0
            while o < w:
                cw = min(TC, w - o)
                offs.append((o, cw))
                o += cw
            return offs

        ahead = CFG.get("ahead", 2)
        for j in range(min(ahead, len(sup))):
            stage1(*sup[j])
        for j in range(len(sup)):
            if j + ahead < len(sup):
                stage1(*sup[j + ahead])
            pb, pl, pw = sup[j]
            for off, cw in chunks_of(pw):
                stage2(pb, pl, off, cw)

    nc.compile()
    return nc


def _chunks():
    """Chunk bounds along T: small chunks at the head (primes the
    PE->scan->PE pipeline sooner) and at the tail (shorter drain)."""
    hs = CFG.get("head_small", 2)
    ts = CFG.get("tail_small", 2)
    bounds = []
    pos = 0
    for _ in range(hs):
        bounds.append((pos, pos + 256))
        pos += 256
    tail_start = T - ts * 256
    while pos < tail_start:
        bounds.append((pos, pos + TC))
        pos += TC
    while pos < T:
        bounds.append((pos, pos + 256))
        pos += 256
    return bounds


def _schedule(stage1, stage2, nch=NCH):
    sched = CFG.get("sched", "pipe1x")
    if sched == "interleave":
        for ic in range(nch):
            for b in range(BPC):
                stage1(b, ic)
            for b in range(BPC):
                stage2(b, ic)
    elif sched == "pipe1":
        for b in range(BPC):
            stage1(b, 0)
            for ic in range(nch - 1):
                stage1(b, ic + 1)
                stage2(b, ic)
            stage2(b, nch - 1)
    elif sched == "pipe1x":
        order = [(b, ic) for b in range(BPC) for ic in range(nch)]
        stage1(*order[0])
        for k in range(len(order) - 1):
            stage1(*order[k + 1])
            stage2(*order[k])
        stage2(*order[-1])
    elif sched == "weave":
        # fine-grained: mm2(k) oblks woven between mm1(k+1) hblk pairs
        order = [(b, ic) for b in range(BPC) for ic in range(nch)]
        stage1(*order[0])
        for k in range(len(order) - 1):
            bn, icn = order[k + 1]
            bp, icp = order[k]
            stage1(bn, icn, hbs=(0, 1))
            stage2(bp, icp, oblks=(0,))
            stage1(bn, icn, hbs=(2, 3))
            stage2(bp, icp, oblks=(1,))
        stage2(*order[-1])
    else:
        for b in range(BPC):
            for ic in range(nch):
                stage1(b, ic)
                stage2(b, ic)


# ------------------------------------------------------------- legacy build --

def _build(mode: str) -> bass.Bass:
    if mode == "hl8":
        return _build_hl8()
    f32 = mybir.dt.float32
    dt_in = {"bf16": mybir.dt.bfloat16, "f32r": mybir.dt.float32r}.get(mode, f32)
    dt_hs = dt_in

    nc = bacc.Bacc(None, target_bir_lowering=False)

    xT = nc.declare_dram_parameter("xT", [BPC, D_IN, T], dt_in, isOutput=False)
    w_inT = nc.declare_dram_parameter("w_inT", [D_IN, HID], dt_in, isOutput=False)
    w_outT = nc.declare_dram_parameter("w_outT", [HID, D_OUT], dt_in, isOutput=False)
    dcols = nc.declare_dram_parameter("dcols", [128, NH], f32, isOutput=False)
    out = nc.declare_dram_parameter("out", [BPC, D_OUT, T], f32, isOutput=True)

    with tile.TileContext(nc) as tc, ExitStack() as ctx:
        const_pool = ctx.enter_context(tc.tile_pool(name="const", bufs=1))
        x_pool = ctx.enter_context(tc.tile_pool(name="xt", bufs=BPC * ND))
        o_pool = ctx.enter_context(tc.tile_pool(name="ot", bufs=8))
        hs_pool = ctx.enter_context(tc.tile_pool(name="hs", bufs=CFG["hs_bufs"]))
        xp_psum = ctx.enter_context(
            tc.tile_pool(name="xp", bufs=CFG["xp_bufs"], space=bass.MemorySpace.PSUM))
        op_psum = ctx.enter_context(
            tc.tile_pool(name="op", bufs=CFG["op_bufs"], space=bass.MemorySpace.PSUM))

        XP_LEN = CFG["x_piece"]
        xt = {}
        for b in range(BPC):
            for dblk in range(ND):
                xt[(b, dblk)] = x_pool.tile([128, T], dt_in, name="xt", tag="xt")

        def load_x(b, dblk, piece):
            psl = slice(piece * XP_LEN, (piece + 1) * XP_LEN)
            nc.sync.dma_start(xt[(b, dblk)][:, psl],
                              xT[b, dblk * 128:(dblk + 1) * 128, psl])

        for dblk in range(ND):
            load_x(0, dblk, 0)
        wi = []
        for dblk in range(ND):
            w = const_pool.tile([128, HID], dt_in, tag=f"wi{dblk}")
            nc.sync.dma_start(w[:], w_inT[dblk * 128:(dblk + 1) * 128, :])
            wi.append(w)
        wo = []
        for hblk in range(NH):
            w = const_pool.tile([128, D_OUT], dt_in, tag=f"wo{hblk}")
            nc.sync.dma_start(w[:], w_outT[hblk * 128:(hblk + 1) * 128, :])
            wo.append(w)
        dc = const_pool.tile([128, NH], f32, tag="dc")
        nc.sync.dma_start(dc[:], dcols[:])
        for piece in range(1, T // XP_LEN):
            for dblk in range(ND):
                load_x(0, dblk, piece)
        for b in range(1, BPC):
            for piece in range(T // XP_LEN):
                for dblk in range(ND):
                    load_x(b, dblk, piece)

        OP = CFG["out_piece"]
        ot = {}
        prev_hs = {}

        def stage1(b, ic):
            tsl = slice(ic * TC, (ic + 1) * TC)
            for hblk in range(NH):
                xp = xp_psum.tile([128, TC], mybir.dt.float32, name="xp", tag="xp")
                for dblk in range(ND):
                    nc.tensor.matmul(
                        xp[:],
                        wi[dblk][:, hblk * 128:(hblk + 1) * 128],
                        xt[(b, dblk)][:, tsl],
                        start=(dblk == 0), stop=(dblk == ND - 1))
                hs = hs_pool.tile([128, TC], dt_hs, name="hs", tag="hs")
                init = (0.0 if ic == 0
                        else prev_hs[(b, ic - 1, hblk)][:, TC - 1:TC])
                nc.vector.tensor_tensor_scan(
                    hs[:], dc[:, hblk:hblk + 1].to_broadcast((128, TC)),
                    xp[:], init,
                    op0=mybir.AluOpType.mult, op1=mybir.AluOpType.add)
                prev_hs[(b, ic, hblk)] = hs

        def stage2(b, ic):
            q, csl = divmod(ic * TC, OP)
            for oblk in range(NO):
                op = op_psum.tile([128, TC], mybir.dt.float32, name="op", tag="op")
                for hblk in range(NH):
                    nc.tensor.matmul(
                        op[:],
                        wo[hblk][:, oblk * 128:(oblk + 1) * 128],
                        prev_hs[(b, ic, hblk)][:],
                        start=(hblk == 0), stop=(hblk == NH - 1))
                if csl == 0:
                    ot[(b, oblk)] = o_pool.tile([128, OP], mybir.dt.float32,
                                                name="ot", tag="ot")
                nc.scalar.copy(ot[(b, oblk)][:, csl:csl + TC], op[:])
                if csl + TC == OP:
                    nc.sync.dma_start(
                        out[b, oblk * 128:(oblk + 1) * 128,
                            q * OP:(q + 1) * OP],
                        ot[(b, oblk)][:])

        _schedule(stage1, stage2)

    nc.compile()
    return nc


# -------------------------------------------------------------- host side ----

def _prep_inputs_hl8(x, W_in, W_h, W_out):
    import ml_dtypes
    e4 = ml_dtypes.float8_e4m3  # IEEE e4m3 (max 240) — matches device fp8e4

    def q8(a):
        return np.asarray(a, e4)

    def d8(a):
        return np.asarray(a, np.float32)

    # sort channels by decay: largest-u channels first (kept bf16 downstream)
    u = np.ascontiguousarray(np.diagonal(np.asarray(W_h, np.float32)))
    perm = np.argsort(-u, kind="stable")
    up = u[perm]
    W_in_p = np.asarray(W_in, np.float32)[perm, :]
    W_out_p = np.asarray(W_out, np.float32)[:, perm]

    xT = np.transpose(np.asarray(x, np.float32), (0, 2, 1))  # [B, D, T]
    x_hi = q8(xT)
    x_lo = q8((xT - d8(x_hi)) * 16.0)
    hi_r = x_hi.reshape(B, ND, 128, T)
    lo_r = x_lo.reshape(B, ND, 128, T)
    # planes: hi-d0, lo-d0, hi-d1, lo-d1
    xq = np.stack([hi_r[:, 0], lo_r[:, 0], hi_r[:, 1], lo_r[:, 1]], axis=2)
    xq = np.ascontiguousarray(xq)  # [B, 128, 4, T]

    wT = W_in_p.T * SWI  # [D, H]
    W_hi = q8(wT)
    W_lo = q8((wT - d8(W_hi)) * 16.0)
    wa = np.stack([W_hi[:128], q8(d8(W_hi[:128]) / 16.0)], axis=1)
    wb = np.stack([W_hi[128:], q8(d8(W_hi[128:]) / 16.0)], axis=1)
    wc = np.stack([q8(d8(W_lo[:128]) / 16.0), q8(d8(W_lo[128:]) / 16.0)], axis=1)
    # pack per h-block, partition-major: wabc[p, hb, j, i, h'] with j in {A,B,C}
    wabc = np.empty((128, NH, 3, 2, 128), dtype=wa.dtype)
    for hb in range(NH):
        hsl = slice(hb * 128, (hb + 1) * 128)
        wabc[:, hb, 0] = wa[:, :, hsl]
        wabc[:, hb, 1] = wb[:, :, hsl]
        wabc[:, hb, 2] = wc[:, :, hsl]
    wabc = np.ascontiguousarray(wabc)

    w_outT = np.ascontiguousarray(
        (W_out_p[:, :N_HOT] * WOS).T).astype(ml_dtypes.bfloat16)
    # cold stationary pair for DoubleRow: [p, i, o] = 16*W_out[o, cold(i*128+p)]
    wo_c = (W_out_p[:, N_HOT:] * WOS).T  # [256 cold, O]
    wo_cold = np.ascontiguousarray(np.stack([q8(wo_c[:128]), q8(wo_c[128:])],
                                            axis=1))

    dcols = np.ascontiguousarray(up.reshape(NH, 128).T, dtype=np.float32)

    in_maps = []
    for c in range(NCORES):
        in_maps.append({
            "xq": np.ascontiguousarray(xq[c * BPC:(c + 1) * BPC]),
            "wabc": wabc,
            "w_outT": w_outT,
            "wo_cold": wo_cold,
            "dcols": dcols,
        })
    return in_maps


def _prep_inputs(x, W_in, W_h, W_out, mode: str):
    if mode == "hl8":
        return _prep_inputs_hl8(x, W_in, W_h, W_out)
    npdt = np.float32
    if mode == "bf16":
        import ml_dtypes
        npdt = ml_dtypes.bfloat16
    xT = np.ascontiguousarray(np.transpose(np.asarray(x, np.float32), (0, 2, 1))).astype(npdt)
    w_inT = np.ascontiguousarray(np.asarray(W_in, np.float32).T).astype(npdt)
    w_outT = np.ascontiguousarray(np.asarray(W_out, np.float32).T).astype(npdt)
    d = np.ascontiguousarray(np.diagonal(np.asarray(W_h, np.float32)))
    dcols = np.ascontiguousarray(d.reshape(NH, 128).T, dtype=np.float32)
    in_maps = []
    for c in range(NCORES):
        in_maps.append({
            "xT": np.ascontiguousarray(xT[c * BPC:(c + 1) * BPC]),
            "w_inT": w_inT,
            "w_outT": w_outT,
            "dcols": dcols,
        })
    return in_maps


def _get_nc(mode: str = MODE_DEFAULT):
    key = (mode, tuple(sorted(CFG.items())))
    if key not in _cache:
        _cache[key] = _build(mode)
    return _cache[key]


def _run(x, W_in, W_h, W_out, mode: str = MODE_DEFAULT, **spmd_kwargs):
    nc = _get_nc(mode)
    in_maps = _prep_inputs(x, W_in, W_h, W_out, mode)
    res = run_bass_kernel_spmd(nc, in_maps, list(range(NCORES)), **spmd_kwargs)
    scale = SWI * WOS if mode == "hl8" else 1.0
    parts = [np.transpose(np.asarray(res.results[c]["out"]).astype(np.float32),
                          (0, 2, 1)) / scale
             for c in range(NCORES)]
    full = np.concatenate(parts, axis=0).astype(np.float32)
    return full, res


def kernel(x, W_in, W_h, W_out):
    out, _ = _run(x, W_in, W_h, W_out)
    return out

